# revision 3
# baseline (speedup 1.0000x reference)
"""Causal multi-head attention on 8 TRN2 NeuronCores.

Problem: B=2, L=2048, H=16, E=64 (f32 in/out). B*H = 32 (batch, head)
slices are data-parallel: 4 slices per core, no cross-core comm.

Per-core algorithm (per slice, all matmul operands bf16, PSUM f32):
  - S^T[m, l] = sum_e K^T[e, m-tile] Q^T[e, l-tile]   (TensorE, 128x128 blocks,
    only causal blocks li >= mi)
  - P^T = exp(S^T / 8)  (ScalarE, batched over multi-bank PSUM groups; no
    max-subtraction needed: |S/8| <= ~6 for randn inputs)
  - diagonal blocks: causal mask applied in-place with gpsimd affine_select
  - O'^T[e, l] += V[m-tile, e|1]^T P^T[m-tile, l]  (TensorE; ones column
    appended to V produces the softmax denominator in row 64)
  - normalize: O^T[e, l] * (1/denom[l]) via reciprocal + partition broadcast
    + vector multiply; output stored as O^T [e, l], untransposed on host.
"""

import numpy as np
import ml_dtypes
from contextlib import ExitStack

import concourse.bass as bass
import concourse.mybir as mybir
import concourse.tile as tile
from concourse import bacc
from concourse.bass_utils import run_bass_kernel_spmd

B, L, H, E = 2, 2048, 16, 64
N_CORES = 8
NS = (B * H) // N_CORES  # slices per core = 4
NT = L // 128  # 16 tiles of 128 along both l and m
SCALE = 0.125  # 1/sqrt(E)
F32 = mybir.dt.float32
BF16 = mybir.dt.bfloat16
BF16NP = ml_dtypes.bfloat16

# unit index of block (mi, li): blocks stored mi-major, li ascending
def _base(mi):
    return 16 * mi - (mi * (mi - 1)) // 2


N_BLOCKS = _base(NT)  # 136

# exp group sizes alternating between a 4-bank and a 3-bank PSUM tile;
# all groups are whole multiples of their bank count so the strided
# activation view never reads unwritten PSUM.
GROUPS = [16, 12, 16, 12, 16, 12, 16, 12, 12, 12]
assert sum(GROUPS) == N_BLOCKS


def _emit_slice(tc, pools, qT, kT, v, outT, s):
    nc = tc.nc
    (io_q, io_k, io_v, pt_pool, ot_pool, bc_pool, r_pool, nm_pool,
     psA, psB, psO) = pools

    qT_sb = io_q.tile([E, L], BF16)
    nc.sync.dma_start(qT_sb[:, :], qT[s])
    kT_sb = io_k.tile([E, L], BF16)
    nc.sync.dma_start(kT_sb[:, :], kT[s])

    # v_sb holds 16 [128, 65] tiles: cols 65t..65t+63 = V rows 128t..,
    # col 65t+64 stays 1.0 (denominator trick)
    v_sb = io_v.tile([128, NT * 65], BF16)
    nc.gpsimd.memset(v_sb[:, :], 1.0)
    v_src = v[s].rearrange("(t p) e -> p t e", p=128)
    v_dst = v_sb.rearrange("p (t x) -> p t x", t=NT, x=65)[:, :, 0:E]
    nc.sync.dma_start(v_dst, v_src)

    pT = pt_pool.tile([128, N_BLOCKS * 128], BF16)

    blocks = [(mi, li) for mi in range(NT) for li in range(mi, NT)]

    # O-window lp becomes ready after this group index (max unit needed):
    # lp0 -> unit 45 (g3), lp1 -> 91 (g6), lp2 -> 121 (g8), lp3 -> 135 (g9)
    o_trigger = {3: 0, 6: 1, 8: 2, 9: 3}

    ot_sb = ot_pool.tile([65, L], F32)

    u = 0
    for gi, gsize in enumerate(GROUPS):
        nb = 4 if gi % 2 == 0 else 3
        pool = psA if gi % 2 == 0 else psB
        qcnt = gsize // nb
        pg = pool.tile([128, nb * 512], F32)
        for j in range(gsize):
            mi, li = blocks[u + j]
            b, q = j % nb, j // nb
            col = 512 * b + 128 * q
            nc.tensor.matmul(
                pg[:, col : col + 128],
                lhsT=kT_sb[:, 128 * mi : 128 * mi + 128],
                rhs=qT_sb[:, 128 * li : 128 * li + 128],
                start=True,
                stop=True,
            )
        # one activation over the whole group, reading slots in emission
        # order (bank-interleaved) and writing pT contiguously
        src = pg.rearrange("p (b q x) -> p q b x", b=nb, q=4, x=128)[:, :qcnt]
        dst = pT[:, 128 * u : 128 * (u + gsize)].rearrange(
            "p (q b x) -> p q b x", q=qcnt, b=nb, x=128
        )
        nc.scalar.activation(dst, src, mybir.ActivationFunctionType.Exp, scale=SCALE)
        # causal mask on diagonal blocks: keep m <= l'
        for j in range(gsize):
            mi, li = blocks[u + j]
            if mi == li:
                seg = pT[:, 128 * (u + j) : 128 * (u + j) + 128]
                nc.gpsimd.affine_select(
                    out=seg,
                    in_=seg,
                    pattern=[[1, 128]],
                    compare_op=mybir.AluOpType.is_ge,
                    fill=0.0,
                    base=0,
                    channel_multiplier=-1,
                )
        u += gsize

        lp = o_trigger.get(gi)
        if lp is not None:
            li_lo, li_hi = 4 * lp, 4 * lp + 3
            po = psO.tile([65, 512], F32)
            mis = list(range(li_hi + 1))
            for idx, mi in enumerate(mis):
                l0 = max(mi, li_lo)
                n = li_hi - l0 + 1
                us = _base(mi) + (l0 - mi)
                nc.tensor.matmul(
                    po[:, 128 * (l0 - li_lo) : 128 * (l0 - li_lo) + 128 * n],
                    lhsT=v_sb[:, 65 * mi : 65 * mi + 65],
                    rhs=pT[:, 128 * us : 128 * (us + n)],
                    start=(idx == 0),
                    stop=(idx == len(mis) - 1),
                    skip_group_check=True,
                )
            nc.vector.tensor_copy(ot_sb[:, 512 * lp : 512 * lp + 512], po[:, :])

    # normalize: rows 0..63 are O^T, row 64 is the denominator
    r_sb = r_pool.tile([1, L], F32)
    nc.vector.reciprocal(r_sb[:, :], ot_sb[64:65, :])
    bc_sb = bc_pool.tile([E, L], F32)
    nc.gpsimd.partition_broadcast(bc_sb[:, :], r_sb[:, :])
    nm_sb = nm_pool.tile([E, L], F32)
    nc.vector.tensor_tensor(
        nm_sb[:, :], ot_sb[0:E, :], bc_sb[:, :], op=mybir.AluOpType.mult
    )
    nc.sync.dma_start(outT[s], nm_sb[:, :])


def _build():
    nc = bacc.Bacc(
        "TRN2",
        target_bir_lowering=False,
        debug=False,
        enable_asserts=True,
        num_devices=N_CORES,
    )
    qT = nc.dram_tensor("qT", [NS, E, L], BF16, kind="ExternalInput").ap()
    kT = nc.dram_tensor("kT", [NS, E, L], BF16, kind="ExternalInput").ap()
    v = nc.dram_tensor("v", [NS, L, E], BF16, kind="ExternalInput").ap()
    outT = nc.dram_tensor("outT", [NS, E, L], F32, kind="ExternalOutput").ap()

    with tile.TileContext(nc) as tc:
        with ExitStack() as ctx:

            def pool(name, bufs, space="SBUF"):
                return ctx.enter_context(
                    tc.tile_pool(name=name, bufs=bufs, space=space)
                )

            pools = (
                pool("io_q", 2),
                pool("io_k", 2),
                pool("io_v", 2),
                pool("pt", 2),
                pool("ot", 2),
                pool("bc", 2),
                pool("r", 2),
                pool("nm", 2),
                pool("psA", 1, "PSUM"),
                pool("psB", 1, "PSUM"),
                pool("psO", 1, "PSUM"),
            )
            for s in range(NS):
                _emit_slice(tc, pools, qT, kT, v, outT, s)

    nc.compile()
    return nc


_NC_CACHE = {}


def _get_nc():
    if "nc" not in _NC_CACHE:
        _NC_CACHE["nc"] = _build()
    return _NC_CACHE["nc"]


def kernel(queries, keys, values, trace=False, tmpdir=None):
    nc = _get_nc()

    # shard: slice g = b*H + h; per-core slices [4c, 4c+4)
    qTf = np.ascontiguousarray(
        queries.transpose(0, 2, 3, 1).reshape(B * H, E, L)
    ).astype(BF16NP)
    kTf = np.ascontiguousarray(
        keys.transpose(0, 2, 3, 1).reshape(B * H, E, L)
    ).astype(BF16NP)
    vf = np.ascontiguousarray(
        values.transpose(0, 2, 1, 3).reshape(B * H, L, E)
    ).astype(BF16NP)

    in_maps = [
        {
            "qT": qTf[NS * c : NS * (c + 1)],
            "kT": kTf[NS * c : NS * (c + 1)],
            "v": vf[NS * c : NS * (c + 1)],
        }
        for c in range(N_CORES)
    ]

    res = run_bass_kernel_spmd(
        nc, in_maps, core_ids=list(range(N_CORES)), trace=trace, tmpdir=tmpdir
    )

    outT = np.concatenate([res.results[c]["outT"] for c in range(N_CORES)], axis=0)
    # outT: [B*H, E, L] -> [B, L, H, E]
    out = outT.reshape(B, H, E, L).transpose(0, 3, 1, 2)
    out = np.ascontiguousarray(out, dtype=np.float32)
    if trace:
        kernel.last_exec_time_ns = res.exec_time_ns
    return out


# revision 5
# speedup vs baseline: 1.2608x; 1.2608x over previous
"""Causal multi-head attention on 8 TRN2 NeuronCores.

Problem: B=2, L=2048, H=16, E=64 (f32 in/out). B*H = 32 (batch, head)
slices are data-parallel: 4 slices per core, no cross-core comm.

Per-core algorithm (per slice, all matmul operands bf16, PSUM f32):
  - S^T[m, l] = sum_e K^T[e, m-tile] Q^T[e, l-tile]   (TensorE, 128x128 blocks,
    only causal blocks li >= mi)
  - P^T = exp(S^T / 8)  (ScalarE, batched over multi-bank PSUM groups; no
    max-subtraction needed: |S/8| <= ~6 for randn inputs)
  - diagonal blocks: causal mask applied in-place with gpsimd affine_select
  - O'^T[e, l] += V[m-tile, e|1]^T P^T[m-tile, l]  (TensorE; ones column
    appended to V produces the softmax denominator in row 64)
  - normalize: O^T[e, l] * (1/denom[l]) via reciprocal + partition broadcast
    + vector multiply; output stored as O^T [e, l], untransposed on host.
"""

import numpy as np
import ml_dtypes
from contextlib import ExitStack

import concourse.bass as bass
import concourse.mybir as mybir
import concourse.tile as tile
from concourse import bacc
from concourse.bass_utils import run_bass_kernel_spmd

B, L, H, E = 2, 2048, 16, 64
N_CORES = 8
NS = (B * H) // N_CORES  # slices per core = 4
NT = L // 128  # 16 tiles of 128 along both l and m
SCALE = 0.125  # 1/sqrt(E)
F32 = mybir.dt.float32
BF16 = mybir.dt.bfloat16
BF16NP = ml_dtypes.bfloat16

# unit index of block (mi, li): blocks stored mi-major, li ascending
def _base(mi):
    return 16 * mi - (mi * (mi - 1)) // 2


N_BLOCKS = _base(NT)  # 136

# exp group sizes alternating between a 4-bank and a 3-bank PSUM tile;
# all groups are whole multiples of their bank count so the strided
# activation view never reads unwritten PSUM.
GROUPS = [16, 12, 16, 12, 16, 12, 16, 12, 12, 12]
assert sum(GROUPS) == N_BLOCKS


def _emit_slice(tc, pools, qT, kT, v, outT, s):
    nc = tc.nc
    (io_q, io_k, io_v, pt_pool, ot_pool, bc_pool, r_pool, nm_pool,
     psA, psB, psO) = pools

    qT_sb = io_q.tile([E, L], BF16)
    nc.sync.dma_start(qT_sb[:, :], qT[s])
    kT_sb = io_k.tile([E, L], BF16)
    nc.sync.dma_start(kT_sb[:, :], kT[s])

    # v_sb holds 16 [128, 65] tiles: cols 65t..65t+63 = V rows 128t..,
    # col 65t+64 stays 1.0 (denominator trick)
    v_sb = io_v.tile([128, NT * 65], BF16)
    nc.gpsimd.memset(v_sb[:, :], 1.0)
    v_src = v[s].rearrange("(t p) e -> p t e", p=128)
    v_dst = v_sb.rearrange("p (t x) -> p t x", t=NT, x=65)[:, :, 0:E]
    nc.sync.dma_start(v_dst, v_src)

    pT = pt_pool.tile([128, N_BLOCKS * 128], BF16)

    # blocks in mi-major order; unit u of block (mi, li) = _base(mi)+(li-mi).
    # PSUM slots are packed 4-per-bank in unit order; S matmuls are maximal
    # runs of consecutive li within one bank (N up to 512, zero garbage).
    blocks = [(mi, li) for mi in range(NT) for li in range(mi, NT)]

    # O-window lp becomes ready after this group index (max unit needed):
    # lp0 -> unit 45 (g3), lp1 -> 91 (g6), lp2 -> 121 (g8), lp3 -> 135 (g9)
    o_trigger = {3: 0, 6: 1, 8: 2, 9: 3}

    u = 0
    for gi, gsize in enumerate(GROUPS):
        nbank = (gsize + 3) // 4
        pool = psA if gi % 2 == 0 else psB
        pg = pool.tile([128, (4 if gi % 2 == 0 else 3) * 512], F32)
        # emit S matmuls as li-runs clipped at bank boundaries
        j = 0
        while j < gsize:
            mi, li = blocks[u + j]
            n = 1
            while (
                j + n < gsize
                and (j + n) % 4 != 0
                and blocks[u + j + n] == (mi, li + n)
            ):
                n += 1
            col = 128 * j
            nc.tensor.matmul(
                pg[:, col : col + 128 * n],
                lhsT=kT_sb[:, 128 * mi : 128 * mi + 128],
                rhs=qT_sb[:, 128 * li : 128 * li + 128 * n],
                start=True,
                stop=True,
            )
            j += n
        nc.scalar.activation(
            pT[:, 128 * u : 128 * (u + gsize)],
            pg[:, : 128 * gsize],
            mybir.ActivationFunctionType.Exp,
            scale=SCALE,
        )
        # causal mask on diagonal blocks: keep m <= l'
        for j in range(gsize):
            mi, li = blocks[u + j]
            if mi == li:
                seg = pT[:, 128 * (u + j) : 128 * (u + j) + 128]
                nc.gpsimd.affine_select(
                    out=seg,
                    in_=seg,
                    pattern=[[1, 128]],
                    compare_op=mybir.AluOpType.is_ge,
                    fill=0.0,
                    base=0,
                    channel_multiplier=-1,
                )
        u += gsize

        lp = o_trigger.get(gi)
        if lp is not None:
            li_lo, li_hi = 4 * lp, 4 * lp + 3
            po = psO.tile([65, 512], F32)
            mis = list(range(li_hi + 1))
            for idx, mi in enumerate(mis):
                l0 = max(mi, li_lo)
                n = li_hi - l0 + 1
                us = _base(mi) + (l0 - mi)
                nc.tensor.matmul(
                    po[:, 128 * (l0 - li_lo) : 128 * (l0 - li_lo) + 128 * n],
                    lhsT=v_sb[:, 65 * mi : 65 * mi + 65],
                    rhs=pT[:, 128 * us : 128 * (us + n)],
                    start=(idx == 0),
                    stop=(idx == len(mis) - 1),
                    skip_group_check=True,
                )
            # normalize this lp chunk: row 64 of po holds the denominator.
            # reciprocal_approx_fast only works on a base-partition-0 AP, so
            # stage the denominator row into a partition-0 tile first.
            d_sb = r_pool.tile([1, 512], F32, tag="den")
            nc.vector.tensor_copy(d_sb[:, :], po[64:65, :])
            r_sb = r_pool.tile([1, 512], F32, tag="rec")
            nc.vector.reciprocal_approx_fast(r_sb[:, :], d_sb[:, :])
            bc_sb = bc_pool.tile([E, 512], F32)
            nc.gpsimd.partition_broadcast(bc_sb[:, :], r_sb[:, :])
            nm_sb = nm_pool.tile([E, 512], F32)
            nc.vector.tensor_tensor(
                nm_sb[:, :],
                po[0:E, :],
                bc_sb[:, :],
                op=mybir.AluOpType.mult,
            )
            nc.sync.dma_start(outT[s][:, 512 * lp : 512 * lp + 512], nm_sb[:, :])


def _build():
    nc = bacc.Bacc(
        "TRN2",
        target_bir_lowering=False,
        debug=False,
        enable_asserts=True,
        num_devices=N_CORES,
    )
    qT = nc.dram_tensor("qT", [NS, E, L], BF16, kind="ExternalInput").ap()
    kT = nc.dram_tensor("kT", [NS, E, L], BF16, kind="ExternalInput").ap()
    v = nc.dram_tensor("v", [NS, L, E], BF16, kind="ExternalInput").ap()
    outT = nc.dram_tensor("outT", [NS, E, L], F32, kind="ExternalOutput").ap()

    with tile.TileContext(nc) as tc:
        with ExitStack() as ctx:

            def pool(name, bufs, space="SBUF"):
                return ctx.enter_context(
                    tc.tile_pool(name=name, bufs=bufs, space=space)
                )

            pools = (
                pool("io_q", 2),
                pool("io_k", 2),
                pool("io_v", 2),
                pool("pt", 2),
                pool("ot", 2),
                pool("bc", 2),
                pool("r", 2),
                pool("nm", 2),
                pool("psA", 1, "PSUM"),
                pool("psB", 1, "PSUM"),
                pool("psO", 1, "PSUM"),
            )
            for s in range(NS):
                _emit_slice(tc, pools, qT, kT, v, outT, s)

    nc.compile()
    return nc


_NC_CACHE = {}


def _get_nc():
    if "nc" not in _NC_CACHE:
        _NC_CACHE["nc"] = _build()
    return _NC_CACHE["nc"]


def kernel(queries, keys, values, trace=False, tmpdir=None):
    nc = _get_nc()

    # shard: slice g = b*H + h; per-core slices [4c, 4c+4)
    qTf = np.ascontiguousarray(
        queries.transpose(0, 2, 3, 1).reshape(B * H, E, L)
    ).astype(BF16NP)
    kTf = np.ascontiguousarray(
        keys.transpose(0, 2, 3, 1).reshape(B * H, E, L)
    ).astype(BF16NP)
    vf = np.ascontiguousarray(
        values.transpose(0, 2, 1, 3).reshape(B * H, L, E)
    ).astype(BF16NP)

    in_maps = [
        {
            "qT": qTf[NS * c : NS * (c + 1)],
            "kT": kTf[NS * c : NS * (c + 1)],
            "v": vf[NS * c : NS * (c + 1)],
        }
        for c in range(N_CORES)
    ]

    res = run_bass_kernel_spmd(
        nc, in_maps, core_ids=list(range(N_CORES)), trace=trace, tmpdir=tmpdir
    )

    outT = np.concatenate([res.results[c]["outT"] for c in range(N_CORES)], axis=0)
    # outT: [B*H, E, L] -> [B, L, H, E]
    out = outT.reshape(B, H, E, L).transpose(0, 3, 1, 2)
    out = np.ascontiguousarray(out, dtype=np.float32)
    if trace:
        kernel.last_exec_time_ns = res.exec_time_ns
    return out


# revision 11
# speedup vs baseline: 1.3372x; 1.0605x over previous
"""Causal multi-head attention on 8 TRN2 NeuronCores.

Problem: B=2, L=2048, H=16, E=64 (f32 in/out). B*H = 32 (batch, head)
slices are data-parallel: 4 slices per core, no cross-core comm.

Per-core algorithm (per slice, all matmul operands bf16, PSUM f32):
  - S^T[m, l] = sum_e K^T[e, m-tile] Q^T[e, l-tile]   (TensorE, 128x128 blocks,
    only causal blocks li >= mi)
  - P^T = exp(S^T / 8)  (ScalarE, batched over multi-bank PSUM groups; no
    max-subtraction needed: |S/8| <= ~6 for randn inputs)
  - diagonal blocks: causal mask applied in-place with gpsimd affine_select
  - O'^T[e, l] += V[m-tile, e|1]^T P^T[m-tile, l]  (TensorE; ones column
    appended to V produces the softmax denominator in row 64)
  - normalize: O^T[e, l] * (1/denom[l]) via reciprocal + partition broadcast
    + vector multiply; output stored as O^T [e, l], untransposed on host.
"""

import numpy as np
import ml_dtypes
from contextlib import ExitStack

import concourse.bass as bass
import concourse.mybir as mybir
import concourse.tile as tile
from concourse import bacc
from concourse.bass_utils import run_bass_kernel_spmd

B, L, H, E = 2, 2048, 16, 64
N_CORES = 8
NS = (B * H) // N_CORES  # slices per core = 4
NT = L // 128  # 16 tiles of 128 along both l and m
SCALE = 0.125  # 1/sqrt(E)
F32 = mybir.dt.float32
BF16 = mybir.dt.bfloat16
BF16NP = ml_dtypes.bfloat16

# unit index of block (mi, li): blocks stored mi-major, li ascending
def _base(mi):
    return 16 * mi - (mi * (mi - 1)) // 2


N_BLOCKS = _base(NT)  # 136


def _plan():
    """Static per-slice schedule.

    S work is organized as runs: run (mi, lp) covers blocks (mi, li) for
    li in [max(mi, 4lp), 4lp+3] — exactly the span consumed by O-window lp.
    Runs are emitted in pairs (even mi -> PE rows 0-63, odd mi -> rows
    64-127) so the two matmuls execute concurrently on disjoint row groups.

    PSUM banks are strictly segregated by parity: even-mi runs fill psA
    group tiles, odd-mi runs fill psB tiles. Matmuls within one parity share
    a PE row group and therefore execute serially in program order, so an
    activation waiting on the last matmul of its group cannot race an
    in-flight matmul from the other parity (those target other banks).
    Short diagonal runs pack pairwise within parity (3+1 and 2+2) so every
    bank is fully written — the activation never reads uninitialized PSUM.
    """
    runs = []  # dicts: mi, lp, l0, n, idx
    for t in range(8):
        for lp in range(t // 2, 4):
            for mi in (2 * t, 2 * t + 1):
                l0 = max(mi, 4 * lp)
                runs.append(
                    {"mi": mi, "lp": lp, "l0": l0, "n": 4 * lp + 4 - l0,
                     "idx": len(runs)}
                )
    # diagonal-run bank sharing partners (same parity, first -> second)
    pair_first = {1: 3, 5: 7, 9: 11, 13: 15, 2: 6, 10: 14}
    second_of = {v: k for k, v in pair_first.items()}
    abanks, bbanks = [], []
    pending = {}
    for r in runs:
        mi, lp = r["mi"], r["lp"]
        banks = abanks if mi % 2 == 0 else bbanks
        diag = lp == mi // 4 and r["n"] < 4
        if diag and mi in pair_first:
            r["pbank"], r["off"] = len(banks), 0
            banks.append(r["n"])
            pending[pair_first[mi]] = r
        elif diag and mi in second_of:
            first = pending.pop(mi)
            r["pbank"], r["off"] = first["pbank"], 128 * first["n"]
            banks[first["pbank"]] += r["n"]
        else:
            r["pbank"], r["off"] = len(banks), 0
            banks.append(r["n"])
    assert not pending
    assert all(b == 4 for b in abanks) and all(b == 4 for b in bbanks)
    na, nb_ = len(abanks), len(bbanks)  # 18, 16

    # groups: psA in chunks of 4 banks, psB in chunks of 3
    a_groups = [min(4, na - i) for i in range(0, na, 4)]
    b_groups = [min(3, nb_ - i) for i in range(0, nb_, 3)]
    nga = len(a_groups)
    group_sizes = a_groups + b_groups
    a_first = [sum(a_groups[:i]) for i in range(nga)]
    b_first = [sum(b_groups[:i]) for i in range(len(b_groups))]

    for r in runs:
        if r["mi"] % 2 == 0:
            g = min(r["pbank"] // 4, nga - 1)
            r["group"] = g
            r["ps_col"] = 512 * (r["pbank"] - a_first[g]) + r["off"]
            r["pt_col"] = 512 * r["pbank"] + r["off"]
        else:
            g = min(r["pbank"] // 3, len(b_groups) - 1)
            r["group"] = nga + g
            r["ps_col"] = 512 * (r["pbank"] - b_first[g]) + r["off"]
            r["pt_col"] = 512 * (na + r["pbank"]) + r["off"]

    group_nruns = [0] * len(group_sizes)
    for r in runs:
        group_nruns[r["group"]] += 1
    # pT column base of each group
    group_pt_base = [512 * a_first[g] if g < nga else 512 * (na + b_first[g - nga])
                     for g in range(len(group_sizes))]

    # group completion order (emission index of last run) -> per-group list
    # of O-windows that become ready once this group's activation is done
    g_last = [max(r["idx"] for r in runs if r["group"] == g)
              for g in range(len(group_sizes))]
    trigger = {}
    for lp in range(4):
        need = {r["group"] for r in runs if r["lp"] == lp}
        g = max(need, key=lambda gg: g_last[gg])
        trigger.setdefault(g, []).append(lp)
    run_by = {(r["mi"], r["lp"]): r for r in runs}
    return runs, group_sizes, group_nruns, trigger, run_by, na + nb_, nga, group_pt_base


(RUNS, GSIZES, GNRUNS, TRIGGER, RUN_BY, NBANKS, NGA, GPTBASE) = _plan()



def _emit_slice(tc, pools, qT, kT, v, outT, s):
    nc = tc.nc
    (io_q, io_k, io_v, pt_pool, bc_pool, r_pool, nm_pool,
     psA, psB, psO) = pools

    # Q^T/K^T duplicated into both partition halves so odd-mi matmuls can
    # run on PE rows 64-127 concurrently with even-mi on rows 0-63.
    qT_sb = io_q.tile([128, L], BF16)
    nc.sync.dma_start(qT_sb[0:E, :], qT[s])
    nc.sync.dma_start(qT_sb[E:128, :], qT[s])
    kT_sb = io_k.tile([128, L], BF16)
    nc.sync.dma_start(kT_sb[0:E, :], kT[s])
    nc.sync.dma_start(kT_sb[E:128, :], kT[s])

    # v_sb holds 16 [128, 65] tiles: cols 65t..65t+63 = V rows 128t..,
    # col 65t+64 stays 1.0 (denominator trick)
    v_sb = io_v.tile([128, NT * 65], BF16)
    nc.gpsimd.memset(v_sb[:, :], 1.0)
    v_src = v[s].rearrange("(t p) e -> p t e", p=128)
    v_dst = v_sb.rearrange("p (t x) -> p t x", t=NT, x=65)[:, :, 0:E]
    nc.sync.dma_start(v_dst, v_src)

    pT = pt_pool.tile([128, NBANKS * 512], BF16)

    gtile = {}
    gdone = [0] * len(GSIZES)

    def emit_o_window(lp):
        li_lo, li_hi = 4 * lp, 4 * lp + 3
        po = psO.tile([65, 512], F32)
        mis = list(range(li_hi + 1))
        for idx, mi in enumerate(mis):
            r = RUN_BY[(mi, lp)]
            nc.tensor.matmul(
                po[:, 128 * (r["l0"] - li_lo) : 128 * (r["l0"] - li_lo) + 128 * r["n"]],
                lhsT=v_sb[:, 65 * mi : 65 * mi + 65],
                rhs=pT[:, r["pt_col"] : r["pt_col"] + 128 * r["n"]],
                start=(idx == 0),
                stop=(idx == len(mis) - 1),
                skip_group_check=True,
            )
        # normalize: row 64 of po is the denominator; reciprocal_approx_fast
        # requires a base-partition-0 AP, so stage the row at partition 0.
        d_sb = r_pool.tile([1, 512], F32, tag="den")
        nc.vector.tensor_copy(d_sb[:, :], po[64:65, :])
        r_sb = r_pool.tile([1, 512], F32, tag="rec")
        nc.vector.reciprocal_approx_fast(r_sb[:, :], d_sb[:, :])
        bc_sb = bc_pool.tile([E, 512], F32)
        nc.gpsimd.partition_broadcast(bc_sb[:, :], r_sb[:, :])
        nm_sb = nm_pool.tile([E, 512], F32)
        nc.vector.tensor_tensor(
            nm_sb[:, :], po[0:E, :], bc_sb[:, :], op=mybir.AluOpType.mult
        )
        nc.sync.dma_start(outT[s][:, 512 * lp : 512 * lp + 512], nm_sb[:, :])

    for r in RUNS:
        g = r["group"]
        if g not in gtile:
            pool = psA if g < NGA else psB
            gtile[g] = pool.tile(
                [128, (4 if g < NGA else 3) * 512], F32, name="pg", tag="pg"
            )
        mi = r["mi"]
        half = slice(0, E) if mi % 2 == 0 else slice(E, 128)
        nc.tensor.matmul(
            gtile[g][:, r["ps_col"] : r["ps_col"] + 128 * r["n"]],
            lhsT=kT_sb[half, 128 * mi : 128 * mi + 128],
            rhs=qT_sb[half, 128 * r["l0"] : 128 * (r["l0"] + r["n"])],
            start=True,
            stop=True,
        )
        gdone[g] += 1
        if gdone[g] == GNRUNS[g]:
            nb = GSIZES[g]
            nc.scalar.activation(
                pT[:, GPTBASE[g] : GPTBASE[g] + 512 * nb],
                gtile[g][:, : 512 * nb],
                mybir.ActivationFunctionType.Exp,
                scale=SCALE,
            )
            # causal mask on diagonal blocks of this group: keep m <= l'
            for rr in RUNS:
                if rr["group"] == g and rr["l0"] == rr["mi"]:
                    seg = pT[:, rr["pt_col"] : rr["pt_col"] + 128]
                    nc.gpsimd.affine_select(
                        out=seg,
                        in_=seg,
                        pattern=[[1, 128]],
                        compare_op=mybir.AluOpType.is_ge,
                        fill=0.0,
                        base=0,
                        channel_multiplier=-1,
                    )
            for lp in TRIGGER.get(g, []):
                emit_o_window(lp)


def _build():
    nc = bacc.Bacc(
        "TRN2",
        target_bir_lowering=False,
        debug=False,
        enable_asserts=True,
        num_devices=N_CORES,
    )
    qT = nc.dram_tensor("qT", [NS, E, L], BF16, kind="ExternalInput").ap()
    kT = nc.dram_tensor("kT", [NS, E, L], BF16, kind="ExternalInput").ap()
    v = nc.dram_tensor("v", [NS, L, E], BF16, kind="ExternalInput").ap()
    outT = nc.dram_tensor("outT", [NS, E, L], F32, kind="ExternalOutput").ap()

    with tile.TileContext(nc) as tc:
        with ExitStack() as ctx:

            def pool(name, bufs, space="SBUF"):
                return ctx.enter_context(
                    tc.tile_pool(name=name, bufs=bufs, space=space)
                )

            pools = (
                pool("io_q", 2),
                pool("io_k", 2),
                pool("io_v", 2),
                pool("pt", 2),
                pool("bc", 2),
                pool("r", 2),
                pool("nm", 2),
                pool("psA", 1, "PSUM"),
                pool("psB", 1, "PSUM"),
                pool("psO", 1, "PSUM"),
            )
            for s in range(NS):
                _emit_slice(tc, pools, qT, kT, v, outT, s)

    nc.compile()
    return nc


_NC_CACHE = {}


def _get_nc():
    if "nc" not in _NC_CACHE:
        _NC_CACHE["nc"] = _build()
    return _NC_CACHE["nc"]


def kernel(queries, keys, values, trace=False, tmpdir=None):
    nc = _get_nc()

    # shard: slice g = b*H + h; per-core slices [4c, 4c+4)
    qTf = np.ascontiguousarray(
        queries.transpose(0, 2, 3, 1).reshape(B * H, E, L)
    ).astype(BF16NP)
    kTf = np.ascontiguousarray(
        keys.transpose(0, 2, 3, 1).reshape(B * H, E, L)
    ).astype(BF16NP)
    vf = np.ascontiguousarray(
        values.transpose(0, 2, 1, 3).reshape(B * H, L, E)
    ).astype(BF16NP)

    in_maps = [
        {
            "qT": qTf[NS * c : NS * (c + 1)],
            "kT": kTf[NS * c : NS * (c + 1)],
            "v": vf[NS * c : NS * (c + 1)],
        }
        for c in range(N_CORES)
    ]

    res = run_bass_kernel_spmd(
        nc, in_maps, core_ids=list(range(N_CORES)), trace=trace, tmpdir=tmpdir
    )

    outT = np.concatenate([res.results[c]["outT"] for c in range(N_CORES)], axis=0)
    # outT: [B*H, E, L] -> [B, L, H, E]
    out = outT.reshape(B, H, E, L).transpose(0, 3, 1, 2)
    out = np.ascontiguousarray(out, dtype=np.float32)
    if trace:
        kernel.last_exec_time_ns = res.exec_time_ns
    return out


# revision 14
# speedup vs baseline: 1.4085x; 1.0533x over previous
"""Causal multi-head attention on 8 TRN2 NeuronCores.

Problem: B=2, L=2048, H=16, E=64 (f32 in/out). B*H = 32 (batch, head)
slices are data-parallel: 4 slices per core, no cross-core comm.

Per-core algorithm (per slice, all matmul operands bf16, PSUM f32):
  - S^T[m, l] = sum_e K^T[e, m-tile] Q^T[e, l-tile]   (TensorE, 128x128 blocks,
    only causal blocks li >= mi)
  - P^T = exp(S^T / 8)  (ScalarE, batched over multi-bank PSUM groups; no
    max-subtraction needed: |S/8| <= ~6 for randn inputs)
  - diagonal blocks: causal mask applied in-place with gpsimd affine_select
  - O'^T[e, l] += V[m-tile, e|1]^T P^T[m-tile, l]  (TensorE; ones column
    appended to V produces the softmax denominator in row 64)
  - normalize: O^T[e, l] * (1/denom[l]) via reciprocal + partition broadcast
    + vector multiply; output stored as O^T [e, l], untransposed on host.
"""

import numpy as np
import ml_dtypes
from contextlib import ExitStack

import concourse.bass as bass
import concourse.mybir as mybir
import concourse.tile as tile
from concourse import bacc
from concourse.bass_utils import run_bass_kernel_spmd

B, L, H, E = 2, 2048, 16, 64
N_CORES = 8
NS = (B * H) // N_CORES  # slices per core = 4
NT = L // 128  # 16 tiles of 128 along both l and m
SCALE = 0.125  # 1/sqrt(E)
F32 = mybir.dt.float32
BF16 = mybir.dt.bfloat16
BF16NP = ml_dtypes.bfloat16

# unit index of block (mi, li): blocks stored mi-major, li ascending
def _base(mi):
    return 16 * mi - (mi * (mi - 1)) // 2


N_BLOCKS = _base(NT)  # 136


def _plan():
    """Static per-slice schedule.

    S work is organized as runs: run (mi, lp) covers blocks (mi, li) for
    li in [max(mi, 4lp), 4lp+3] — exactly the span consumed by O-window lp.
    Runs are emitted in pairs (even mi -> PE rows 0-63, odd mi -> rows
    64-127) so the two matmuls execute concurrently on disjoint row groups.

    PSUM banks are strictly segregated by parity: even-mi runs fill psA
    group tiles, odd-mi runs fill psB tiles. Matmuls within one parity share
    a PE row group and therefore execute serially in program order, so an
    activation waiting on the last matmul of its group cannot race an
    in-flight matmul from the other parity (those target other banks).
    Short diagonal runs pack pairwise within parity (3+1 and 2+2) so every
    bank is fully written — the activation never reads uninitialized PSUM.
    """
    runs = []  # dicts: mi, lp, l0, n, idx
    for t in range(8):
        for lp in range(t // 2, 4):
            for mi in (2 * t, 2 * t + 1):
                l0 = max(mi, 4 * lp)
                runs.append(
                    {"mi": mi, "lp": lp, "l0": l0, "n": 4 * lp + 4 - l0,
                     "idx": len(runs)}
                )
    # diagonal-run bank sharing partners (same parity, first -> second)
    pair_first = {1: 3, 5: 7, 9: 11, 13: 15, 2: 6, 10: 14}
    second_of = {v: k for k, v in pair_first.items()}
    abanks, bbanks = [], []
    pending = {}
    for r in runs:
        mi, lp = r["mi"], r["lp"]
        banks = abanks if mi % 2 == 0 else bbanks
        diag = lp == mi // 4 and r["n"] < 4
        if diag and mi in pair_first:
            r["pbank"], r["off"] = len(banks), 0
            banks.append(r["n"])
            pending[pair_first[mi]] = r
        elif diag and mi in second_of:
            first = pending.pop(mi)
            r["pbank"], r["off"] = first["pbank"], 128 * first["n"]
            banks[first["pbank"]] += r["n"]
        else:
            r["pbank"], r["off"] = len(banks), 0
            banks.append(r["n"])
    assert not pending
    assert all(b == 4 for b in abanks) and all(b == 4 for b in bbanks)
    na, nb_ = len(abanks), len(bbanks)  # 18, 16

    # groups: psA in chunks of 4 banks, psB in chunks of 3
    a_groups = [min(4, na - i) for i in range(0, na, 4)]
    b_groups = [min(2, nb_ - i) for i in range(0, nb_, 2)]
    nga = len(a_groups)
    group_sizes = a_groups + b_groups
    a_first = [sum(a_groups[:i]) for i in range(nga)]
    b_first = [sum(b_groups[:i]) for i in range(len(b_groups))]

    for r in runs:
        if r["mi"] % 2 == 0:
            g = min(r["pbank"] // 4, nga - 1)
            r["group"] = g
            r["ps_col"] = 512 * (r["pbank"] - a_first[g]) + r["off"]
            r["pt_col"] = 512 * r["pbank"] + r["off"]
        else:
            g = min(r["pbank"] // 2, len(b_groups) - 1)
            r["group"] = nga + g
            r["ps_col"] = 512 * (r["pbank"] - b_first[g]) + r["off"]
            r["pt_col"] = 512 * (na + r["pbank"]) + r["off"]

    group_nruns = [0] * len(group_sizes)
    for r in runs:
        group_nruns[r["group"]] += 1
    # pT column base of each group
    group_pt_base = [512 * a_first[g] if g < nga else 512 * (na + b_first[g - nga])
                     for g in range(len(group_sizes))]

    # group completion order (emission index of last run) -> per-group list
    # of O-windows that become ready once this group's activation is done
    g_last = [max(r["idx"] for r in runs if r["group"] == g)
              for g in range(len(group_sizes))]
    trigger = {}
    for lp in range(4):
        need = {r["group"] for r in runs if r["lp"] == lp}
        g = max(need, key=lambda gg: g_last[gg])
        trigger.setdefault(g, []).append(lp)
    run_by = {(r["mi"], r["lp"]): r for r in runs}
    return runs, group_sizes, group_nruns, trigger, run_by, na + nb_, nga, group_pt_base


(RUNS, GSIZES, GNRUNS, TRIGGER, RUN_BY, NBANKS, NGA, GPTBASE) = _plan()



def _emit_slice(tc, pools, qT, kT, v, outT, s):
    nc = tc.nc
    (io_q, io_k, io_v, pt_pool, bc_pool, r_pool, nm_pool,
     psA, psB, psO) = pools

    # Q^T/K^T duplicated into both partition halves so odd-mi matmuls can
    # run on PE rows 64-127 concurrently with even-mi on rows 0-63.
    qT_sb = io_q.tile([128, L], BF16)
    nc.sync.dma_start(qT_sb[0:E, :], qT[s])
    nc.sync.dma_start(qT_sb[E:128, :], qT[s])
    kT_sb = io_k.tile([128, L], BF16)
    nc.sync.dma_start(kT_sb[0:E, :], kT[s])
    nc.sync.dma_start(kT_sb[E:128, :], kT[s])

    # v_sb holds 16 [128, 65] tiles: cols 65t..65t+63 = V rows 128t..,
    # col 65t+64 stays 1.0 (denominator trick)
    v_sb = io_v.tile([128, NT * 65], BF16)
    nc.gpsimd.memset(v_sb[:, :], 1.0)
    v_src = v[s].rearrange("(t p) e -> p t e", p=128)
    v_dst = v_sb.rearrange("p (t x) -> p t x", t=NT, x=65)[:, :, 0:E]
    nc.sync.dma_start(v_dst, v_src)

    pT = pt_pool.tile([128, NBANKS * 512], BF16)

    gtile = {}
    gdone = [0] * len(GSIZES)

    def emit_o_window(lp):
        li_lo, li_hi = 4 * lp, 4 * lp + 3
        po = psO.tile([65, 512], F32)
        mis = list(range(li_hi + 1))
        for idx, mi in enumerate(mis):
            r = RUN_BY[(mi, lp)]
            nc.tensor.matmul(
                po[:, 128 * (r["l0"] - li_lo) : 128 * (r["l0"] - li_lo) + 128 * r["n"]],
                lhsT=v_sb[:, 65 * mi : 65 * mi + 65],
                rhs=pT[:, r["pt_col"] : r["pt_col"] + 128 * r["n"]],
                start=(idx == 0),
                stop=(idx == len(mis) - 1),
                skip_group_check=True,
            )
        # normalize: row 64 of po is the denominator; reciprocal_approx_fast
        # requires a base-partition-0 AP, so stage the row at partition 0.
        d_sb = r_pool.tile([1, 512], F32, tag="den")
        nc.vector.tensor_copy(d_sb[:, :], po[64:65, :])
        r_sb = r_pool.tile([1, 512], F32, tag="rec")
        nc.vector.reciprocal_approx_fast(r_sb[:, :], d_sb[:, :])
        bc_sb = bc_pool.tile([E, 512], F32)
        nc.gpsimd.partition_broadcast(bc_sb[:, :], r_sb[:, :])
        nm_sb = nm_pool.tile([E, 512], F32)
        nc.vector.tensor_tensor(
            nm_sb[:, :], po[0:E, :], bc_sb[:, :], op=mybir.AluOpType.mult
        )
        nc.sync.dma_start(outT[s][:, 512 * lp : 512 * lp + 512], nm_sb[:, :])

    for r in RUNS:
        g = r["group"]
        if g not in gtile:
            pool = psA if g < NGA else psB
            gtile[g] = pool.tile(
                [128, (4 if g < NGA else 2) * 512], F32, name="pg", tag="pg"
            )
        mi = r["mi"]
        half = slice(0, E) if mi % 2 == 0 else slice(E, 128)
        nc.tensor.matmul(
            gtile[g][:, r["ps_col"] : r["ps_col"] + 128 * r["n"]],
            lhsT=kT_sb[half, 128 * mi : 128 * mi + 128],
            rhs=qT_sb[half, 128 * r["l0"] : 128 * (r["l0"] + r["n"])],
            start=True,
            stop=True,
        )
        gdone[g] += 1
        if gdone[g] == GNRUNS[g]:
            nb = GSIZES[g]
            nc.scalar.activation(
                pT[:, GPTBASE[g] : GPTBASE[g] + 512 * nb],
                gtile[g][:, : 512 * nb],
                mybir.ActivationFunctionType.Exp,
                scale=SCALE,
            )
            # causal mask on diagonal blocks of this group: keep m <= l'
            for rr in RUNS:
                if rr["group"] == g and rr["l0"] == rr["mi"]:
                    seg = pT[:, rr["pt_col"] : rr["pt_col"] + 128]
                    nc.gpsimd.affine_select(
                        out=seg,
                        in_=seg,
                        pattern=[[1, 128]],
                        compare_op=mybir.AluOpType.is_ge,
                        fill=0.0,
                        base=0,
                        channel_multiplier=-1,
                    )
            for lp in TRIGGER.get(g, []):
                emit_o_window(lp)


def _build():
    nc = bacc.Bacc(
        "TRN2",
        target_bir_lowering=False,
        debug=False,
        enable_asserts=True,
        num_devices=N_CORES,
    )
    qT = nc.dram_tensor("qT", [NS, E, L], BF16, kind="ExternalInput").ap()
    kT = nc.dram_tensor("kT", [NS, E, L], BF16, kind="ExternalInput").ap()
    v = nc.dram_tensor("v", [NS, L, E], BF16, kind="ExternalInput").ap()
    outT = nc.dram_tensor("outT", [NS, E, L], F32, kind="ExternalOutput").ap()

    with tile.TileContext(nc) as tc:
        with ExitStack() as ctx:

            def pool(name, bufs, space="SBUF"):
                return ctx.enter_context(
                    tc.tile_pool(name=name, bufs=bufs, space=space)
                )

            pools = (
                pool("io_q", 2),
                pool("io_k", 2),
                pool("io_v", 2),
                pool("pt", 2),
                pool("bc", 2),
                pool("r", 2),
                pool("nm", 2),
                pool("psA", 1, "PSUM"),
                pool("psB", 1, "PSUM"),
                pool("psO", 2, "PSUM"),
            )
            for s in range(NS):
                _emit_slice(tc, pools, qT, kT, v, outT, s)

    nc.compile()
    return nc


_NC_CACHE = {}


def _get_nc():
    if "nc" not in _NC_CACHE:
        _NC_CACHE["nc"] = _build()
    return _NC_CACHE["nc"]


def kernel(queries, keys, values, trace=False, tmpdir=None):
    nc = _get_nc()

    # shard: slice g = b*H + h; per-core slices [4c, 4c+4)
    qTf = np.ascontiguousarray(
        queries.transpose(0, 2, 3, 1).reshape(B * H, E, L)
    ).astype(BF16NP)
    kTf = np.ascontiguousarray(
        keys.transpose(0, 2, 3, 1).reshape(B * H, E, L)
    ).astype(BF16NP)
    vf = np.ascontiguousarray(
        values.transpose(0, 2, 1, 3).reshape(B * H, L, E)
    ).astype(BF16NP)

    in_maps = [
        {
            "qT": qTf[NS * c : NS * (c + 1)],
            "kT": kTf[NS * c : NS * (c + 1)],
            "v": vf[NS * c : NS * (c + 1)],
        }
        for c in range(N_CORES)
    ]

    res = run_bass_kernel_spmd(
        nc, in_maps, core_ids=list(range(N_CORES)), trace=trace, tmpdir=tmpdir
    )

    outT = np.concatenate([res.results[c]["outT"] for c in range(N_CORES)], axis=0)
    # outT: [B*H, E, L] -> [B, L, H, E]
    out = outT.reshape(B, H, E, L).transpose(0, 3, 1, 2)
    out = np.ascontiguousarray(out, dtype=np.float32)
    if trace:
        kernel.last_exec_time_ns = res.exec_time_ns
    return out
